# revision 22
# baseline (speedup 1.0000x reference)
"""GCN VGAE encoder (3x GCNConv) on 8 Trainium2 NeuronCores.

Strategy: shard nodes across 8 cores, partition edges by destination
node (host-side 1D graph partitioning), replicate weights.

Math: A_hat = D^-1/2 (A+I) D^-1/2, dinv = 1/sqrt(deg+1). Aggregation
commutes with the linear projections, so layer 1 aggregates RAW x rows
(gathered bf16 from a replicated DRAM table -- no AllGather needed) and
projects afterwards. The per-edge norm w_e = dinv[src]*dinv[dst] is
folded into the one-hot scatter weights (host-computed), and the self
loop contributes dinv^2 * row analytically (host-precomputed scaled
transposed x). Layer-1 chunk matmuls run "transposed" (lhsT=messages,
rhs=one-hot) so the aggregate lands feature-major: the projections then
use stationary weights and fused bias+relu, with a single PE transpose
per tile to get z2 back to node-major for the gather table. Layers 2/3
share the adjacency, so W_mu|W_log fuse into one 64-wide pass over the
z2 table, stored bf16 padded to 128 cols (gather elements must be 256B
and 256B-strided) so both passes share one int16 index space over the
per-shard padded tables and gathers feed PE directly in bf16. One-hot
matrices are built 16 chunks per DVE instruction via stride-0 broadcast
APs. Gather calls round-robin over 4 SWDGE queues (a single queue
drains at ~1 SDMA engine's bandwidth); z2/output stores are grouped 8
tiles per DMA to keep SDMA slots free for gathers.
"""

import numpy as np
import ml_dtypes

P = 128
BF = ml_dtypes.bfloat16


def _ceil_div(a, b):
    return -(-a // b)


class _Plan:
    """Host-side edge partitioning shared by all cores (SPMD => one
    common padded chunk structure = max over cores).

    Edges are grouped by (dst core, src quarter, dst tile); each (q,t)
    run is padded to a multiple of 128 so chunks never span tiles.
    Quarter q's edge stream is the concatenation of its runs over t.
    """

    def __init__(self, n, n_cores, cpc, src, dst):
        assert n % n_cores == 0
        self.n = n
        self.n_cores = n_cores
        self.cpc = cpc
        self.S = n // n_cores
        self.T = _ceil_div(self.S, P)
        self.SPAD = self.T * P
        # shards per quarter: largest divisor of n_cores with
        # spq*SPAD <= int16 range (gather idx are int16)
        spq = n_cores
        while spq > 1 and spq * self.SPAD > 32767:
            spq //= 2
        assert spq * self.SPAD <= 32767
        self.SPQ = spq
        self.NQ = n_cores // spq
        self.QR = spq * self.SPAD                    # padded rows/quarter
        self.NP = n_cores * self.SPAD                # padded table rows

        core = dst // self.S
        drel = dst - core * self.S
        tt = drel // P
        loc = (drel % P).astype(np.float32)
        src_core = src // self.S
        q = src_core // spq
        qsrc = ((src_core % spq) * self.SPAD
                + (src - src_core * self.S)).astype(np.int16)

        T, NQ = self.T, self.NQ
        counts = np.bincount((core * NQ + q) * T + tt,
                             minlength=n_cores * NQ * T).reshape(
            n_cores, NQ, T)
        rl = counts.max(axis=0)                       # [NQ, T] run lengths
        # runs are packed back-to-back (no 128-padding); chunks may span
        # adjacent tiles, each (chunk, tile) overlap gets its own one-hot
        # column ("cp" column).
        qt_start = np.zeros((NQ, T), np.int64)
        qt_start[:, 1:] = np.cumsum(rl, axis=1)[:, :-1]
        self.qt_start = qt_start
        self.NQC = _ceil_div(rl.sum(axis=1), P)       # chunks per quarter
        self.NCH = int(self.NQC.sum())
        self.qcol0 = np.zeros(NQ + 1, np.int64)
        self.qcol0[1:] = np.cumsum(self.NQC)

        # cp columns in (q, chunk, tile) order => contiguous per call
        ncalls = [_ceil_div(int(c), cpc) for c in self.NQC]
        j0 = qt_start // P
        j1 = (qt_start + np.maximum(rl, 1) - 1) // P  # last chunk of run
        self.tile_ops = [[] for _ in range(T)]        # t -> [(q, j, cp)]
        self.cp0_call = {}
        keys = []
        cp = 0
        for qq in range(NQ):
            per_j = [[] for _ in range(int(self.NQC[qq]))]
            for t in range(T):
                if rl[qq, t] == 0:
                    continue
                for j in range(int(j0[qq, t]), int(j1[qq, t]) + 1):
                    per_j[j].append(t)
            for j, tl in enumerate(per_j):
                if j % cpc == 0:
                    self.cp0_call[(qq, j // cpc)] = cp
                for t in tl:
                    self.tile_ops[t].append((qq, j, cp))
                    keys.append((qq * 100000 + j) * T + t)
                    cp += 1
            self.cp0_call[(qq, ncalls[qq])] = cp
        self.NCP = cp
        self.cp_keys = np.asarray(keys, np.int64)
        self.MAXCOLS = max(
            (self.cp0_call[(qq, k + 1)] - self.cp0_call[(qq, k)])
            for qq in range(NQ) for k in range(ncalls[qq])) if cp else cpc

        # order edges by (core, quarter, tile, src) -- src-sorted runs
        # give the gather DMA ascending addresses (DRAM locality).
        sidx = np.lexsort((src, tt, q, core))
        self.sc = core[sidx]
        self.sq = q[sidx]
        self.st = tt[sidx]
        self.sqsrc = qsrc[sidx]
        self.sloc = loc[sidx]
        self.ssrc = src[sidx]
        self.sdst = dst[sidx]
        gkey = (self.sc * NQ + self.sq) * T + self.st
        first = np.r_[True, gkey[1:] != gkey[:-1]]
        gstart = np.flatnonzero(first)
        glen = np.diff(np.r_[gstart, len(gkey)])
        self.rank = np.arange(len(gkey)) - np.repeat(gstart, glen)

        self.ncalls = [_ceil_div(int(c), cpc) for c in self.NQC]
        self.IDXCOLS = self.NCH * 8

    def core_arrays(self, c, dinv):
        """Per-core tensors: gather idx [P, IDXCOLS] i16, dl [P, NCH]
        bf16, w [P, NCH] bf16, dinv2 [SPAD] f32."""
        NQ, NCH = self.NQ, self.NCH
        m = self.sc == c
        mq = self.sq[m]
        mt = self.st[m]
        mrank = self.rank[m]
        msrc = self.sqsrc[m]
        mloc = self.sloc[m]
        mw = (dinv[self.ssrc[m]] * dinv[self.sdst[m]]).astype(np.float32)

        pos = self.qt_start[mq, mt] + mrank           # slot in q stream
        key = (mq * 100000 + pos // P) * self.T + mt
        cp = np.searchsorted(self.cp_keys, key)
        assert np.array_equal(self.cp_keys[cp], key)
        dl = np.full((self.NCP, P), 255.0, np.float32)
        w = np.zeros((self.NCP, P), np.float32)
        dl[cp, pos % P] = mloc
        w[cp, pos % P] = mw

        idx_out = np.zeros((P, self.IDXCOLS), np.int16)
        for qq in range(NQ):
            nqc = int(self.NQC[qq])
            if nqc == 0:
                continue
            arr = np.zeros(nqc * P, np.int16)
            mm = mq == qq
            arr[pos[mm]] = msrc[mm]
            c0 = int(self.qcol0[qq]) * 8
            wrapped = arr.reshape(nqc * 8, 16).T      # [16, nqc*8]
            idx_out[:, c0:c0 + nqc * 8] = np.tile(wrapped, (8, 1))

        d2 = np.zeros(self.SPAD, np.float32)
        d2[:self.S] = dinv[c * self.S:(c + 1) * self.S] ** 2
        return (idx_out, dl.T.astype(BF).copy(), w.T.astype(BF).copy(), d2)


def _build(plan, d_in, d_h, d_o):
    """Build the SPMD Bass program (same for every core)."""
    import concourse.mybir as mybir
    import concourse.tile as tile
    from concourse import bacc
    from concourse.masks import make_identity
    from contextlib import ExitStack
    import os

    F32 = mybir.dt.float32
    BF16 = mybir.dt.bfloat16
    I16 = mybir.dt.int16
    AF = mybir.ActivationFunctionType
    OP = mybir.AluOpType
    n, T, NQ, SPAD, S, QR = plan.n, plan.T, plan.NQ, plan.SPAD, plan.S, plan.QR
    cpc = plan.cpc
    n_cores = plan.n_cores
    _stage = int(os.environ.get("GCN_STAGE", "2"))
    _nocoll = bool(os.environ.get("GCN_NOCOLL"))
    _nq = int(os.environ.get("GCN_QUEUES", "4"))

    nc = bacc.Bacc("TRN2", target_bir_lowering=False,
                   debug=False, num_devices=n_cores, num_swdge_queues=_nq)

    NP = plan.NP
    xtab_d = nc.dram_tensor("xtab", [NP, d_in], BF16, kind="ExternalInput")
    xst_d = nc.dram_tensor("xst", [P, T * P], BF16, kind="ExternalInput")
    w1_d = nc.dram_tensor("w1", [d_in, d_h], BF16, kind="ExternalInput")
    wc_d = nc.dram_tensor("wcat", [d_h, d_o], BF16, kind="ExternalInput")
    b1_d = nc.dram_tensor("b1", [d_h, 1], F32, kind="ExternalInput")
    bc_d = nc.dram_tensor("bcat", [1, d_o], BF16, kind="ExternalInput")
    d2_d = nc.dram_tensor("dinv2", [P, T], F32, kind="ExternalInput")
    dl_d = nc.dram_tensor("dl", [P, plan.NCP], BF16, kind="ExternalInput")
    w_d = nc.dram_tensor("w", [P, plan.NCP], BF16, kind="ExternalInput")
    idx_d = nc.dram_tensor("gidx", [P, plan.IDXCOLS], I16,
                           kind="ExternalInput")
    out_d = nc.dram_tensor("out2", [P, T * d_o], BF16, kind="ExternalOutput")

    z2s_own = nc.dram_tensor("z2s_own", [SPAD, P], BF16, kind="Internal")
    z2s_full = nc.dram_tensor("z2s_full", [NP, P], BF16, kind="Internal",
                              addr_space="Shared")
    rg = [list(range(n_cores))]

    qctr = [0]

    def next_queue():
        q = qctr[0] % _nq
        qctr[0] += 1
        return q

    with tile.TileContext(nc, num_cores=n_cores) as tc, ExitStack() as st:
        cp = st.enter_context(tc.tile_pool(name="consts", bufs=1))
        bigp = st.enter_context(tc.tile_pool(name="big", bufs=1))
        gp = st.enter_context(tc.tile_pool(name="gath", bufs=5))
        outp = st.enter_context(tc.tile_pool(name="outs", bufs=2))
        ohp = st.enter_context(tc.tile_pool(name="oh", bufs=2))
        tp = st.enter_context(tc.tile_pool(name="stage", bufs=4))
        mmp = st.enter_context(tc.tile_pool(name="mm", bufs=4, space="PSUM"))
        aggp = st.enter_context(tc.tile_pool(name="agg", bufs=4,
                                             space="PSUM"))

        # ---- constants ----
        iota_i = cp.tile([P, P], mybir.dt.int32)
        nc.gpsimd.iota(iota_i[:], pattern=[[1, P]], base=0,
                       channel_multiplier=0)
        iota_b = cp.tile([P, P], BF16)
        nc.vector.tensor_copy(iota_b[:], iota_i[:])
        ident_b = cp.tile([P, P], BF16)
        make_identity(nc, ident_b[:])
        ones_row = cp.tile([1, P], BF16)
        nc.gpsimd.memset(ones_row[:], 1.0)
        zeros_t = cp.tile([P, d_in], F32)
        nc.gpsimd.memset(zeros_t[:], 0.0)

        w1_sb = cp.tile([d_in, d_h], BF16)
        nc.sync.dma_start(w1_sb[:], w1_d[:, :])
        wc_sb = cp.tile([d_h, d_o], BF16)
        nc.sync.dma_start(wc_sb[:], wc_d[:, :])
        b1c = cp.tile([d_h, 1], F32)
        nc.sync.dma_start(b1c[:], b1_d[:, :])
        bcr = cp.tile([1, d_o], BF16)
        nc.sync.dma_start(bcr[:], bc_d[:, :])
        d2_sb = cp.tile([P, T], F32)
        nc.sync.dma_start(d2_sb[:], d2_d[:, :])
        dl_sb = cp.tile([P, plan.NCP], BF16)
        nc.sync.dma_start(dl_sb[:], dl_d[:, :])
        w_sb = cp.tile([P, plan.NCP], BF16)
        nc.sync.dma_start(w_sb[:], w_d[:, :])
        idx_sb = cp.tile([P, plan.IDXCOLS], I16)
        nc.sync.dma_start(idx_sb[:], idx_d[:, :])

        # bcat broadcast to all partitions via ones-matmul
        psb = mmp.tile([P, P], F32, space="PSUM", tag="mm", bufs=3)
        nc.tensor.matmul(psb[:, :d_o], lhsT=ones_row[:], rhs=bcr[:],
                         start=True, stop=True)
        bcbc = cp.tile([P, d_o], F32)
        nc.vector.tensor_copy(bcbc[:], psb[:, :d_o])

        xst_sb = bigp.tile([P, T, P], BF16)
        nc.sync.dma_start(xst_sb[:, :, :], xst_d[:, :])
        z2s_sb = bigp.tile([P, T, d_o], BF16)

        MAXC = plan.MAXCOLS

        def oh_build(qq, k):
            """Weighted one-hot columns for all (chunk, tile) parts of
            call (qq, k): oh[p, c, d] = (d == dl[p, col]) * w[p, col]."""
            c0 = plan.cp0_call[(qq, k)]
            nco = plan.cp0_call[(qq, k + 1)] - c0
            oh = ohp.tile([P, MAXC, P], BF16, tag=f"oh{qq}")
            iota_bc = iota_b[:].unsqueeze(1).broadcast_to([P, nco, P])
            dl_bc = dl_sb[:, c0:c0 + nco].unsqueeze(2).broadcast_to(
                [P, nco, P])
            w_bc = w_sb[:, c0:c0 + nco].unsqueeze(2).broadcast_to(
                [P, nco, P])
            nc.vector.tensor_tensor(oh[:, :nco, :], iota_bc, dl_bc,
                                    OP.is_equal)
            nc.vector.tensor_tensor(oh[:, :nco, :], oh[:, :nco, :], w_bc,
                                    OP.mult)
            return oh

        def gather_call(table, qq, k, d_f, dtype):
            L = min(cpc, int(plan.NQC[qq]) - k * cpc)
            i0 = (int(plan.qcol0[qq]) + k * cpc) * 8
            q0 = qq * QR
            q1 = q0 + QR
            g = gp.tile([P, cpc, d_f], dtype, tag=f"g{qq}")
            nc.gpsimd.dma_gather(
                out_ap=g[:, :L, :],
                in_ap=table[q0:q1, :],
                idxs_ap=idx_sb[:, i0:i0 + L * 8],
                num_idxs=L * P,
                num_idxs_reg=L * P,
                elem_size=d_f,
                single_packet=False,
                queue_num=next_queue(),
            )
            return g, L

        def tile_ops(t):
            return plan.tile_ops[t]

        # ---- pass 1 (transposed): psum1T[f, d] = sum_e x[src_e]w_e ----
        def tile1(t, psa):
            agg1T = tp.tile([P, P], BF16, tag="aggT")
            if psa is not None:
                nc.vector.tensor_tensor(agg1T[:], psa[:],
                                        xst_sb[:, t, :], OP.add)
            else:
                nc.vector.tensor_copy(agg1T[:], xst_sb[:, t, :])
            psh = mmp.tile([P, P], F32, space="PSUM", tag="mm", bufs=3)
            nc.tensor.matmul(psh[:d_h, :], lhsT=w1_sb[:], rhs=agg1T[:],
                             start=True, stop=True)
            hT = tp.tile([d_h, P], BF16, tag="hT")
            nc.scalar.activation(hT[:], psh[:d_h, :], AF.Relu, bias=b1c[:])
            psz = mmp.tile([P, P], F32, space="PSUM", tag="mm", bufs=3)
            nc.tensor.matmul(psz[:d_o, :], lhsT=wc_sb[:], rhs=hT[:],
                             start=True, stop=True)
            z2T = tp.tile([d_o, P], BF16, tag="z2T")
            nc.vector.tensor_copy(z2T[:], psz[:d_o, :])
            psn = mmp.tile([P, P], BF16, space="PSUM", tag="mmb", bufs=2)
            nc.tensor.transpose(psn[:, :d_o], z2T[:], ident_b[:d_o, :d_o])
            nc.vector.tensor_copy(z2s_sb[:, t, :], psn[:, :d_o])
            if t % 8 == 7 or t == T - 1:
                t0 = (t // 8) * 8
                dst_ap = z2s_own[t0 * P:(t + 1) * P, :d_o].rearrange(
                    "(t p) f -> p t f", p=P)
                nc.sync.dma_start(dst_ap, z2s_sb[:, t0:t + 1, :])

        if _stage in (1, 2):
            calls = {}
            for t in range(T):
                ops = tile_ops(t)
                if not ops:
                    tile1(t, None)
                    continue
                psa = aggp.tile([P, P], F32, space="PSUM", tag="agg",
                                bufs=3)
                for i, (qq, j, cpcol) in enumerate(ops):
                    k = j // cpc
                    if (qq, k) not in calls:
                        g, L = gather_call(xtab_d, qq, k, d_in, BF16)
                        calls[(qq, k)] = (g, oh_build(qq, k))
                    g, oh = calls[(qq, k)]
                    nc.tensor.matmul(
                        psa[:], lhsT=g[:, j - k * cpc, :],
                        rhs=oh[:, cpcol - plan.cp0_call[(qq, k)], :],
                        start=(i == 0),
                        stop=(i == len(ops) - 1))
                tile1(t, psa)
        else:
            for t in range(T):
                tile1(t, None)

        if _nocoll:
            nc.sync.dma_start(z2s_full[0:SPAD, :], z2s_own[:, :])
        else:
            nc.gpsimd.collective_compute(
                "AllGather", mybir.AluOpType.bypass, replica_groups=rg,
                ins=[z2s_own[:, :].opt()], outs=[z2s_full[:, :].opt()])

        # ---- pass 2: out[t] = bcat + sum w_e z2[src] + dinv2 z2own ----
        og_hold = [None]

        def tile2(t, psa):
            if t % 8 == 0:
                og_hold[0] = outp.tile([P, 8, d_o], BF16, tag="og", name="og")
            og = og_hold[0]
            nc.vector.scalar_tensor_tensor(
                og[:, t % 8, :], z2s_sb[:, t, :], d2_sb[:, t:t + 1],
                psa[:] if psa is not None else zeros_t[:, :d_o],
                OP.mult, OP.add)
            nc.vector.tensor_tensor(og[:, t % 8, :], og[:, t % 8, :],
                                    bcbc[:], OP.add)
            if t % 8 == 7 or t == T - 1:
                t0 = (t // 8) * 8
                nc.sync.dma_start(out_d[:, t0 * d_o:(t + 1) * d_o],
                                  og[:, :t - t0 + 1, :])

        if _stage in (2, 3):
            calls = {}
            for t in range(T):
                ops = tile_ops(t)
                if not ops:
                    tile2(t, None)
                    continue
                psa = aggp.tile([P, d_o], F32, space="PSUM", tag="agg",
                                bufs=3)
                for i, (qq, j, cpcol) in enumerate(ops):
                    k = j // cpc
                    if (qq, k) not in calls:
                        g, L = gather_call(z2s_full, qq, k, P, BF16)
                        calls[(qq, k)] = (g, oh_build(qq, k))
                    g, oh = calls[(qq, k)]
                    nc.tensor.matmul(
                        psa[:], lhsT=oh[:, cpcol - plan.cp0_call[(qq, k)], :],
                        rhs=g[:, j - k * cpc, :d_o],
                        start=(i == 0),
                        stop=(i == len(ops) - 1))
                tile2(t, psa)
        else:
            for t in range(T):
                tile2(t, None)

    nc.compile()
    return nc


_CACHE = {}


def _get_program(n, e, d_in, d_h, d_o, n_cores, cpc, edge_key, src, dst):
    key = (n, e, d_in, d_h, d_o, n_cores, cpc, edge_key)
    if key not in _CACHE:
        plan = _Plan(n, n_cores, cpc, src, dst)
        nc = _build(plan, d_in, d_h, d_o)
        _CACHE[key] = (plan, nc)
    return _CACHE[key]


def kernel(x, edge_index, W1, b1, W_mu, b_mu, W_log, b_log,
           n_cores=8, cpc=16, _run_kwargs=None):
    from concourse.bass_utils import run_bass_kernel_spmd

    x = np.asarray(x, np.float32)
    edge_index = np.asarray(edge_index)
    W1 = np.asarray(W1, np.float32)
    Wcat = np.concatenate([np.asarray(W_mu, np.float32),
                           np.asarray(W_log, np.float32)], axis=1)
    bcat = np.concatenate([np.asarray(b_mu, np.float32),
                           np.asarray(b_log, np.float32)])
    b1 = np.asarray(b1, np.float32)
    n, d_in = x.shape
    d_h = W1.shape[1]
    d_o = Wcat.shape[1]
    lat = np.asarray(W_mu, np.float32).shape[1]
    src = edge_index[0].astype(np.int64)
    dst = edge_index[1].astype(np.int64)

    edge_key = hash((src.tobytes(), dst.tobytes()))
    plan, nc = _get_program(n, len(src), d_in, d_h, d_o, n_cores, cpc,
                            edge_key, src, dst)

    deg = np.bincount(dst, minlength=n)
    dinv = (1.0 / np.sqrt(1.0 + deg)).astype(np.float32)
    xtab = np.zeros((plan.NP, d_in), BF)
    xv = x.astype(BF)
    for c in range(n_cores):
        xtab[c * plan.SPAD:c * plan.SPAD + plan.S] = \
            xv[c * plan.S:(c + 1) * plan.S]
    in_maps = []
    for c in range(n_cores):
        idx_u, dl, w, d2 = plan.core_arrays(c, dinv)
        # xst[f, t*P + p] = dinv2[t*P+p] * x[c*S + t*P + p, f]
        xpad = np.zeros((plan.SPAD, d_in), np.float32)
        xpad[:plan.S] = x[c * plan.S:(c + 1) * plan.S]
        xst = (xpad * d2[:, None]).T.astype(BF).copy()
        in_maps.append({
            "xtab": xtab, "xst": xst,
            "w1": W1.astype(BF), "wcat": Wcat.astype(BF),
            "b1": b1.astype(np.float32)[:, None],
            "bcat": bcat.astype(BF)[None, :],
            "dinv2": d2.reshape(plan.T, P).T.copy(),
            "dl": dl, "w": w, "gidx": idx_u,
        })

    global _LAST_RESULT, _LAST_IN_MAPS
    _LAST_IN_MAPS = in_maps
    res = run_bass_kernel_spmd(nc, in_maps, core_ids=list(range(n_cores)),
                               **(_run_kwargs or {}))
    _LAST_RESULT = res
    outs = []
    for c in range(n_cores):
        o = np.asarray(res.results[c]["out2"]).astype(np.float32)
        o = o.reshape(P, plan.T, -1)
        outs.append(o.transpose(1, 0, 2).reshape(plan.SPAD, -1)[:plan.S])
    out = np.concatenate(outs, axis=0)
    return (out[:, :lat].copy(), out[:, lat:].copy())


_LAST_RESULT = None
_LAST_IN_MAPS = None
